# revision 33
# baseline (speedup 1.0000x reference)
"""Causal multi-head attention on 8 trn2 NeuronCores (v2).

Sharding: head-parallel. Each core owns 2 of the 16 heads (128 of the 1024
channels) for all 4 batches. Per core:
  Q^T/K^T/V^T projections (local 128 channels) from x^T (host-transposed
  bf16), flash-style causal attention in score-transposed layout S^T[k, q],
  softmax denominators ride as ones columns appended to V (PV matmul lands
  den on its own PSUM partition); normalization applied to A^T via a rank-2
  "R" matmul built from reciprocals; local Wo row-block matmul produces a
  [8192, 1024] bf16 partial per core, summed (+bias) on host.

v2 changes vs baseline:
  - bf16 tensors end-to-end (PSUM accumulation stays f32): halves DMA
    bytes / DVE work / SBUF, kills the fp32r 4x penalty at free dim < 256.
  - Attention inner loop software-pipelined with 2-iteration lookahead:
    per iteration emit QK(k), exp(k), PV(k-2), fillers, so PV never heads
    the PE queue before its exp has had ~2 iterations to complete. Keeps
    the PE gapless (p-state ramp to 2.4 GHz needs continuous busy).
  - Causal diagonal mask applied *after* exp by zeroing the upper triangle
    of pt with gpsimd.affine_select (idle engine) instead of a DVE
    trimask-add between QK and exp.
  - Softmax reciprocal: denominators staged [2,512]->[128,8] with 2 DMAs
    (16B runs) instead of 16 scatter DMAs, reciprocal on DVE, 2 DMAs back
    (straight into an f32r tile - same bits, no recast copy).
  - PV PSUM copied directly into aT (bf16) - no stgA/scalar staging.
  - x loaded with 8 coarse DMAs per batch (32 fine for batch 0) instead
    of 32; weights DMA'd straight into bf16 tiles (host pre-arranged).
"""
import sys

sys.path.insert(0, "/opt/trn_rl_repo")

import numpy as np
import ml_dtypes

import concourse.bass as bass
import concourse.tile as tile
from concourse import bacc, mybir
from concourse.bass_utils import run_bass_kernel_spmd

f32 = mybir.dt.float32
f32r = mybir.dt.float32r
bf16 = mybir.dt.bfloat16
EXP = mybir.ActivationFunctionType.Exp

B, S, D, H, HD = 4, 2048, 1024, 16, 64
NCORES = 8
CLOC = D // NCORES       # 128 local channels = 2 heads per core
BS = B * S               # 8192
QT = 4                   # q tiles of 512 per batch
KB = 16                  # k blocks of 128 per batch


def build_program():
    nc = bacc.Bacc("TRN2", target_bir_lowering=False, debug=False)

    xtr_d = nc.dram_tensor("xtr", [D, BS], bf16, kind="ExternalInput").ap()
    wq_d = nc.dram_tensor("wq", [128, D], bf16, kind="ExternalInput").ap()
    wk_d = nc.dram_tensor("wk", [128, D], bf16, kind="ExternalInput").ap()
    wv_d = nc.dram_tensor("wv", [128, D], bf16, kind="ExternalInput").ap()
    wo_d = nc.dram_tensor("wo", [CLOC, D], bf16, kind="ExternalInput").ap()
    selc_d = nc.dram_tensor("selc", [66, CLOC], f32, kind="ExternalInput").ap()
    out_d = nc.dram_tensor("out", [BS, D], bf16, kind="ExternalOutput").ap()

    with tile.TileContext(nc) as tc:
        _Builder(nc, tc, xtr_d, wq_d, wk_d, wv_d, wo_d, selc_d, out_d).build()
    nc.compile()
    return nc


class _Builder:
    def __init__(self, nc, tc, xtr_d, wq_d, wk_d, wv_d, wo_d, selc_d, out_d):
        self.nc = nc
        self.tc = tc
        self.xtr_d = xtr_d
        self.w_d = {"q": wq_d, "k": wk_d, "v": wv_d}
        self.wo_d = wo_d
        self.selc_d = selc_d
        self.out_d = out_d
        self.st_b = {}
        from collections import deque
        self.fillers = deque()
        self.hi_fillers = deque()
        self.n_enq = 0
        self.n_drained = 0
        self.markers = {}

    # -------------------------------------------------- filler machinery
    # two deques: hi (Wo thunks - release aT slots promptly, breaking the
    # aT -> pv -> PE-FIFO dependency cycle) drains before the main one
    def _enq(self, thunk, hi=False):
        if hi:
            self.hi_fillers.append(thunk)
        else:
            self.fillers.append(thunk)
            self.n_enq += 1

    def _mark(self, key):
        self.markers[key] = self.n_enq

    def _drain_n(self, budget):
        while self.hi_fillers and budget > 0:
            self.hi_fillers.popleft()()
            budget -= 1
        while self.fillers and budget > 0:
            self.fillers.popleft()()
            self.n_drained += 1
            budget -= 1

    def _drain_until(self, key):
        tgt = self.markers.get(key, 0)
        while self.n_drained < tgt:
            self.fillers.popleft()()
            self.n_drained += 1

    def _drain_all(self):
        while self.hi_fillers:
            self.hi_fillers.popleft()()
        while self.fillers:
            self.fillers.popleft()()
            self.n_drained += 1

    # -------------------------------------------------- top level
    def build(self):
        from contextlib import ExitStack

        nc, tc = self.nc, self.tc
        with ExitStack() as ctx:
            p = self.p = {}
            for name, bufs, space in (
                ("consts", 1, None), ("wpool", 1, None), ("xtp", 2, None),
                ("qkv", 2, None), ("vtpool", 1, None), ("vpp", 28, None),
                ("ptp", 6, None), ("atp", 2, None), ("denp", 2, None),
                ("outp", 2, None),
                ("ps_st", 2, "PSUM"), ("ps_pv", 2, "PSUM"),
                ("ps_a", 2, "PSUM"),
            ):
                kw = {"space": space} if space else {}
                p[name] = ctx.enter_context(
                    tc.tile_pool(name=name, bufs=bufs, **kw))

            # issue batch 0's first q-tile of x before everything else so
            # the first projection isn't gated on 18 earlier DMA issues
            self._xt_dma(0, fine=True, qts=(0,))
            self._consts()
            self._weights()
            self._xt_dma(0, fine=True, qts=(1, 2, 3))

            # flat (b, qt) stream; den_norm/Wo lag one qtile behind and wrap
            # across batch boundaries so qt3's den chain hides behind the
            # next batch's attention instead of stalling the PE
            prev = None
            for b in range(B):
                if b == 0:
                    self._enqueue_proj(0)
                if b + 1 < B:
                    self._xt_dma(b + 1, fine=False)
                    self._enqueue_proj(b + 1)
                for qt in range(QT):
                    self._drain_until(("proj", b, qt))
                    if prev is not None:
                        self._den_recip(*prev)
                    self._attention_qtile(b, qt)
                    self._den_part1(b, qt)
                    if prev is not None:
                        self._den_norm(*prev)
                        self._enqueue_wo(*prev)
                    prev = (b, qt)
            self._den_recip(*prev, fast=True)
            self._den_norm(*prev)
            self._enqueue_wo(*prev)
            self._drain_all()

    # -------------------------------------------------- setup
    def _consts(self):
        nc, p = self.nc, self.p
        ident = p["consts"].tile([128, 128], bf16)
        nc.gpsimd.memset(ident[:], 0.0)
        nc.gpsimd.affine_select(
            out=ident[:], in_=ident[:],
            compare_op=mybir.AluOpType.not_equal, fill=1.0, base=0,
            pattern=[[-1, 128]], channel_multiplier=1,
        )
        # selc carries the head-select rows at partitions 0:2 and again at
        # 64:66 (for the fast final-qtile den path, whose R matmul rhs is
        # dens rows 64:66 and needs a base-partition-aligned lhsT)
        sel_stg = p["consts"].tile([66, 128], f32)
        nc.sync.dma_start(sel_stg[0:2, :], self.selc_d[0:2, :])
        nc.sync.dma_start(sel_stg[64:66, :], self.selc_d[64:66, :])
        sel = p["consts"].tile([2, 128], f32r)
        nc.vector.tensor_copy(sel[:], sel_stg[0:2, :])
        sel64 = p["consts"].tile([66, 128], f32r)
        nc.vector.tensor_copy(sel64[64:66, :], sel_stg[64:66, :])
        self.sel64 = sel64
        ones_c = p["consts"].tile([128, 1], bf16)
        nc.vector.memset(ones_c[:], 1.0)
        self.ident, self.sel, self.ones_c = ident, sel, ones_c

    def _weights(self):
        # split each weight into 4 column-chunk DMAs so they spread across
        # DMA queues and the first projection isn't gated on one 256KB DMA
        nc, p = self.nc, self.p
        self.w_sb = {}
        for name in ("v", "q", "k"):
            w_sb = p["wpool"].tile([128, D], bf16, tag="w_" + name,
                                   name="w_" + name)
            for i in range(4):
                cs = slice(i * 256, (i + 1) * 256)
                nc.sync.dma_start(w_sb[:, cs], self.w_d[name][:, cs])
            self.w_sb[name] = w_sb
        self.wo_sb = p["wpool"].tile([128, D], bf16, tag="w_o", name="w_o")
        for i in range(4):
            cs = slice(i * 256, (i + 1) * 256)
            nc.sync.dma_start(self.wo_sb[:, cs], self.wo_d[:, cs])

    def _st(self, b):
        return self.st_b.setdefault(b, {})

    # -------------------------------------------------- x load
    def _xt_dma(self, b, fine, qts=None):
        nc, p = self.nc, self.p
        st = self._st(b)
        if "xt" not in st:
            st["xt"] = p["xtp"].tile([128, 8 * S], bf16, tag="xt", name="xt")
        xt = st["xt"]
        if fine:
            for qt in (range(QT) if qts is None else qts):
                for dc in range(8):
                    nc.sync.dma_start(
                        xt[:, dc * S + qt * 512: dc * S + (qt + 1) * 512],
                        self.xtr_d[dc * 128:(dc + 1) * 128,
                                   b * S + qt * 512: b * S + (qt + 1) * 512])
        else:
            for dc in range(8):
                nc.sync.dma_start(
                    xt[:, dc * S:(dc + 1) * S],
                    self.xtr_d[dc * 128:(dc + 1) * 128,
                               b * S:(b + 1) * S])

    # -------------------------------------------------- projections
    def _enqueue_proj(self, b):
        nc, p = self.nc, self.p
        st = self._st(b)
        st["qT"] = p["qkv"].tile([128, S], bf16, tag="qT", name="qT")
        st["kT"] = p["qkv"].tile([128, S], bf16, tag="kT", name="kT")
        st["vT"] = p["vtpool"].tile([128, S], bf16, tag="vT", name="vT")
        st["v_tiles"] = [None] * KB
        xt = st["xt"]
        for qt in range(QT):
            for name in ("v", "q", "k"):
                dst = st[{"q": "qT", "k": "kT", "v": "vT"}[name]]

                box = {}

                def mk_mm(dc, name=name, box=box, qt=qt, xt=xt):
                    def thunk():
                        if dc == 0:
                            box["pps"] = p["ps_a"].tile(
                                [128, 512], f32, tag="a", name="pps")
                        nc.tensor.matmul(
                            box["pps"][:],
                            self.w_sb[name][:, dc * 128:(dc + 1) * 128],
                            xt[:, dc * S + qt * 512: dc * S + (qt + 1) * 512],
                            start=(dc == 0), stop=(dc == 7))
                    return thunk

                for dc in range(8):
                    self._enq(mk_mm(dc))

                def cp(dst=dst, box=box, qt=qt):
                    nc.vector.tensor_copy(
                        dst[:, qt * 512:(qt + 1) * 512], box["pps"][:])

                self._enq(cp)
                if name == "v":
                    for kb in range(4 * qt, 4 * qt + 4):
                        self._enq(
                            lambda kb=kb, b=b: self._vtrans_one(b, kb))
            self._mark(("proj", b, qt))

    def _vtrans_one(self, b, kb):
        nc, p = self.nc, self.p
        st = self._st(b)
        vT = st["vT"]
        tp2 = p["ps_a"].tile([128, 512], bf16, tag="a", name="tp2")
        nc.tensor.transpose(
            tp2[:, 0:128], vT[:, kb * 128:(kb + 1) * 128], self.ident[:])
        vt = p["vpp"].tile([128, 131], bf16, tag="vp", name="vt")
        # [V_A(0:64) | 1(64) | V_B(65:129) | pad(129) | 1(130)]
        nc.vector.memset(vt[:, 64:65], 1.0)
        nc.vector.memset(vt[:, 129:131], 1.0)
        nc.vector.tensor_copy(vt[:, 0:64], tp2[:, 0:64])
        nc.vector.tensor_copy(vt[:, 65:129], tp2[:, 64:128])
        st["v_tiles"][kb] = vt

    # -------------------------------------------------- attention
    def _attention_qtile(self, b, qt):
        nc, p = self.nc, self.p
        st = self._st(b)
        qT, kT, v_tiles = st["qT"], st["kT"], st["v_tiles"]
        if "aT" not in st:
            st["aT"] = p["atp"].tile([128, S], bf16, tag="aT", name="aT")
        pvA = p["ps_pv"].tile([128, 512], f32, tag="pv", name="pvA")
        pvB = p["ps_pv"].tile([128, 512], f32, tag="pv", name="pvB")
        st["pv"] = (pvA, pvB)
        nkb = 4 * qt + 4
        pts = {}
        for it in range(nkb + 2):
            if it < nkb:
                kb = it
                off = max(0, (kb - 4 * qt) * 128)
                stp = p["ps_st"].tile([128, 1024], f32, tag="st", name="stp")
                nc.tensor.matmul(
                    stp[:, off:512], kT[0:64, kb * 128:(kb + 1) * 128],
                    qT[0:64, qt * 512 + off:(qt + 1) * 512],
                    start=True, stop=True)
                nc.tensor.matmul(
                    stp[:, 512 + off:1024],
                    kT[64:128, kb * 128:(kb + 1) * 128],
                    qT[64:128, qt * 512 + off:(qt + 1) * 512],
                    start=True, stop=True)
                pt = p["ptp"].tile([128, 1024], bf16, tag="pt", name="pt")
                st_v = stp[:].rearrange("p (h q) -> p h q", h=2)[:, :, off:512]
                pt_v = pt[:].rearrange("p (h q) -> p h q", h=2)[:, :, off:512]
                nc.scalar.activation(pt_v, st_v, EXP, scale=0.125)
                if kb >= 4 * qt:
                    # zero the invalid (key > query) upper triangle post-exp
                    tri = pt[:].rearrange("p (h q) -> p h q", h=2)[
                        :, :, off:off + 128]
                    nc.gpsimd.affine_select(
                        out=tri, in_=tri,
                        compare_op=mybir.AluOpType.is_ge, fill=0.0, base=0,
                        pattern=[[0, 2], [1, 128]], channel_multiplier=-1)
                pts[kb] = pt
            j = it - 2
            if 0 <= j < nkb:
                offj = max(0, (j - 4 * qt) * 128)
                ptj = pts.pop(j)
                nc.tensor.matmul(
                    pvA[0:65, offj:512], v_tiles[j][:, 0:65],
                    ptj[:, offj:512],
                    start=(j == 0), stop=(j == nkb - 1))
                nc.tensor.matmul(
                    pvB[0:66, offj:512], v_tiles[j][:, 65:131],
                    ptj[:, 512 + offj:1024],
                    start=(j == 0), stop=(j == nkb - 1))
            self._drain_n(3)

    # -------------------------------------------------- denominators
    def _den_part1(self, b, qt):
        nc, p = self.nc, self.p
        st = self._st(b)
        aT = st["aT"]
        pvA, pvB = st.pop("pv")
        cols = slice(qt * 512, (qt + 1) * 512)
        # attention out -> aT (bf16); head B staged then DMA'd to rows 64:128.
        # aT copy on ACT, stgB on DVE: pv PSUM released by parallel queues.
        nc.scalar.copy(aT[0:64, cols], pvA[0:64, :])
        stgB = p["denp"].tile([64, 512], bf16, tag="stgB", name="stgB")
        nc.vector.tensor_copy(stgB[:], pvB[0:64, :])
        nc.sync.dma_start(aT[64:128, cols], stgB[:])
        # denominator rows (f32 bits, f32r-tagged) -> SBUF, repartition [128, 8]
        dens = p["denp"].tile([66, 512], f32r, tag="dens", name="dens")
        nc.vector.tensor_copy(dens[64:66, :], pvB[64:66, :])
        nc.vector.tensor_copy(dens[64:65, :], pvA[64:65, :])
        densP = p["denp"].tile([128, 8], f32r, tag="densP", name="densP")
        nc.sync.dma_start(
            densP[:, 0:4],
            dens[64:65, :].rearrange("o (p q) -> o p q", q=4))
        nc.sync.dma_start(
            densP[:, 4:8],
            dens[65:66, :].rearrange("o (p q) -> o p q", q=4))
        st.setdefault("den_pend", {})[qt] = (dens, densP)

    def _den_recip(self, b, qt, fast=False):
        # early half: reciprocal + scatter-back, emitted at the next qtile's
        # start so the DVE recip isn't queued behind that qtile's copies.
        # fast path (final qtile): recip directly on [2, 512] - costs more
        # DVE cycles but skips two DMA hops nothing would hide.
        nc, p = self.nc, self.p
        st = self._st(b)
        dens, densP = st["den_pend"].pop(qt)
        with nc.allow_low_precision(reason="f32r recip: same bits as f32"):
            if fast:
                nc.vector.reciprocal(dens[64:66, :], dens[64:66, :])
                st.setdefault("recip_pend", {})[qt] = ("dens", dens)
                return
            nc.vector.reciprocal(densP[:], densP[:])
        recip_rr = p["denp"].tile([2, 512], f32r, tag="recip", name="recip_rr")
        nc.sync.dma_start(
            recip_rr[0:1, :].rearrange("o (p q) -> o p q", q=4),
            densP[:, 0:4])
        nc.sync.dma_start(
            recip_rr[1:2, :].rearrange("o (p q) -> o p q", q=4),
            densP[:, 4:8])
        st.setdefault("recip_pend", {})[qt] = ("rr", recip_rr)

    def _den_norm(self, b, qt):
        nc, p = self.nc, self.p
        st = self._st(b)
        aT = st["aT"]
        kind, rtile = st["recip_pend"].pop(qt)
        cols = slice(qt * 512, (qt + 1) * 512)
        r_ps = p["ps_a"].tile([128, 512], f32, tag="a", name="r_ps")
        if kind == "dens":
            nc.tensor.matmul(r_ps[:], self.sel64[64:66, :], rtile[64:66, :],
                             start=True, stop=True)
        else:
            nc.tensor.matmul(r_ps[:], self.sel[:], rtile[:],
                             start=True, stop=True)
        with nc.allow_low_precision(reason="bf16 attention output is in-budget"):
            nc.vector.tensor_mul(aT[0:64, cols], aT[0:64, cols],
                                 r_ps[0:64, :])
            nc.vector.tensor_mul(aT[64:128, cols], aT[64:128, cols],
                                 r_ps[64:128, :])

    # -------------------------------------------------- output projection
    def _enqueue_wo(self, b, qt):
        nc, p = self.nc, self.p
        aT = self._st(b)["aT"]
        for qb in range(4 * qt, 4 * qt + 4):
            def thunk(qb=qb, aT=aT, b=b):
                o_sb = p["outp"].tile([128, 1024], bf16, tag="osb",
                                      name="o_sb")
                for nt in range(2):
                    pout = p["ps_a"].tile([128, 512], f32, tag="a",
                                          name="pout")
                    nc.tensor.matmul(
                        pout[:], aT[:, qb * 128:(qb + 1) * 128],
                        self.wo_sb[:, nt * 512:(nt + 1) * 512],
                        start=True, stop=True)
                    if (qb + nt) % 2 == 0:
                        nc.vector.tensor_copy(
                            o_sb[:, nt * 512:(nt + 1) * 512], pout[:])
                    else:
                        nc.scalar.copy(
                            o_sb[:, nt * 512:(nt + 1) * 512], pout[:])
                nc.sync.dma_start(
                    self.out_d[b * S + qb * 128: b * S + (qb + 1) * 128, :],
                    o_sb[:])
            self._enq(thunk, hi=True)


_PROGRAM_CACHE = {}


def _get_program():
    if "nc" not in _PROGRAM_CACHE:
        _PROGRAM_CACHE["nc"] = build_program()
    return _PROGRAM_CACHE["nc"]


def make_in_maps(x, Wq, Wk, Wv, Wo):
    bf = ml_dtypes.bfloat16
    x_flat = np.asarray(x, dtype=np.float32).reshape(BS, D)
    xtr = np.ascontiguousarray(x_flat.T).astype(bf)
    sel_const = np.zeros((66, CLOC), dtype=np.float32)
    for r0 in (0, 64):
        sel_const[r0 + 0, 0:64] = 1.0
        sel_const[r0 + 1, 64:128] = 1.0

    def warr(W, sl):
        # [p, dc*128 + j] = W[dc*128 + p, sl][j]
        w = np.asarray(W, dtype=np.float32)[:, sl]
        return np.ascontiguousarray(
            w.reshape(8, 128, 128).transpose(1, 0, 2).reshape(128, D)
        ).astype(bf)

    maps = []
    for c in range(NCORES):
        sl = slice(c * CLOC, (c + 1) * CLOC)
        maps.append({
            "xtr": xtr,
            "wq": warr(Wq, sl),
            "wk": warr(Wk, sl),
            "wv": warr(Wv, sl),
            "wo": np.ascontiguousarray(
                np.asarray(Wo, dtype=np.float32)[sl, :]).astype(bf),
            "selc": sel_const,
        })
    return maps


def run(x, Wq, Wk, Wv, Wo, bo, trace=False, **kw):
    nc = _get_program()
    maps = make_in_maps(x, Wq, Wk, Wv, Wo)
    res = run_bass_kernel_spmd(nc, maps, core_ids=list(range(NCORES)),
                               trace=trace, **kw)
    acc = np.asarray(res.results[0]["out"]).astype(np.float32)
    for c in range(1, NCORES):
        acc = acc + np.asarray(res.results[c]["out"]).astype(np.float32)
    out = (acc + np.asarray(bo, dtype=np.float32)).reshape(B, S, D)
    return out, res


def kernel(x, Wq, Wk, Wv, Wo, bo):
    out, _ = run(x, Wq, Wk, Wv, Wo, bo, trace=False)
    return out
